# revision 7
# baseline (speedup 1.0000x reference)
"""Trainium2 Bass kernel for nn_AdvancedNeuroplasticityLayer.

Reference computation (B=64, I=2048, O=2048, S=10, C=512):
  conn_mask = (|weight| * connection_strength > pruning_threshold)
  eff_w     = weight * conn_mask                                   [O, I]
  ctx       = mean(context, -1)                                    [B]
  astro     = sigmoid(ctx[:,None] * astrocyte_activation[None,:])  [B, O]
  astro    *= (astro > astrocyte_threshold)
  mean_astro = mean(astro, 0)                                      [O]
  synaptic  = x @ (eff_w * mean_astro[:,None]).T + bias            [B, O]
  seg_act   = einsum('bi,ois->bos', x, dendrite_segments)          [B, O, S]
  dendritic = sum_s relu(seg_act) * sigmoid(dendritic_gates)       [B, O]
  out       = relu(synaptic + dendritic)

Sharding: output dim O split across 8 cores (256 outputs each); x/context
replicated.  Host pre-transposes weight-like tensors to [ktile, 128, n]
(contraction dim I on partitions) so all DMAs are large & contiguous.
Matmuls run as float32r (fp32 bits at bf16 streaming rate).
"""

import os
import sys

import numpy as np

for _p in ("/opt/trn_rl_repo", "/root/.axon_site/_ro/trn_rl_repo"):
    if os.path.isdir(_p) and _p not in sys.path:
        sys.path.insert(0, _p)

B = 64
I = 2048
O = 2048
S = 10
C = 512
NCORES = 8
OS = O // NCORES  # 256 outputs per core
P = 128
KT = I // P  # 16 k-tiles
NW = 512  # matmul moving-dim chunk
NCH = (OS * S) // NW  # 5 psum chunks of the (s, o) dendritic columns

_BUILT = None


def _build():
    import concourse.tile as tile
    from concourse import bacc, mybir
    from concourse.alu_op_type import AluOpType

    f32 = mybir.dt.float32
    f32r = mybir.dt.float32r

    nc = bacc.Bacc("TRN2", target_bir_lowering=False, debug=False, num_devices=NCORES)

    d_xtr = nc.dram_tensor("xtr", [KT, P, B], f32r, kind="ExternalInput").ap()
    d_xtf = nc.dram_tensor("xtf", [KT, P, B], f32, kind="ExternalInput").ap()
    d_ctx = nc.dram_tensor("ctx", [B, C], f32, kind="ExternalInput").ap()
    d_wt = nc.dram_tensor("wt", [KT, P, OS], f32, kind="ExternalInput").ap()
    d_cst = nc.dram_tensor("cst", [KT, P, OS], f32, kind="ExternalInput").ap()
    d_seg = nc.dram_tensor("seg", [NCH, KT, P, NW], f32r, kind="ExternalInput").ap()
    d_gates = nc.dram_tensor("gates", [1, S * OS], f32, kind="ExternalInput").ap()
    d_bias = nc.dram_tensor("bias", [1, OS], f32, kind="ExternalInput").ap()
    d_aa = nc.dram_tensor("aa", [1, OS], f32, kind="ExternalInput").ap()
    d_at = nc.dram_tensor("at", [1, OS], f32, kind="ExternalInput").ap()
    d_thr = nc.dram_tensor("thr", [1, 1], f32, kind="ExternalInput").ap()
    d_out = nc.dram_tensor("out", [B, OS], f32, kind="ExternalOutput").ap()

    with tile.TileContext(nc) as tc:
        with (
            tc.tile_pool(name="const", bufs=1) as const,
            tc.tile_pool(name="wgt", bufs=1) as wgt,
            tc.tile_pool(name="segp", bufs=2) as segp,
            tc.tile_pool(name="gtp", bufs=3) as gtp,
            tc.tile_pool(name="tmp", bufs=2) as tmp,
            tc.tile_pool(name="pd", bufs=2, space="PSUM") as pdp,
            tc.tile_pool(name="psmall", bufs=1, space="PSUM") as psmall,
        ):
            # ---- small input loads ----
            x_tr = const.tile([P, KT, B], f32r)
            nc.sync.dma_start(out=x_tr[:], in_=d_xtr.rearrange("k p b -> p k b"))
            x_tf = const.tile([P, KT, B], f32)
            nc.sync.dma_start(out=x_tf[:], in_=d_xtf.rearrange("k p b -> p k b"))
            ctx_t = const.tile([B, C], f32)
            nc.sync.dma_start(out=ctx_t[:], in_=d_ctx[:])
            aa_r = const.tile([B, OS], f32)
            nc.gpsimd.dma_start(out=aa_r[:], in_=d_aa.to_broadcast([B, OS]))
            at_r = const.tile([B, OS], f32)
            nc.gpsimd.dma_start(out=at_r[:], in_=d_at.to_broadcast([B, OS]))
            bias_r = const.tile([B, OS], f32)
            nc.gpsimd.dma_start(out=bias_r[:], in_=d_bias.to_broadcast([B, OS]))
            thr_c = const.tile([P, 1], f32)
            nc.gpsimd.dma_start(out=thr_c[:], in_=d_thr.to_broadcast([P, 1]))
            gb = const.tile([B, S * OS], f32)
            nc.gpsimd.dma_start(out=gb[:], in_=d_gates.to_broadcast([B, S * OS]))

            ones64 = const.tile([B, 1], f32)
            nc.vector.memset(ones64[:], 1.0)
            ones1 = const.tile([1, B], f32)
            nc.vector.memset(ones1[:], 1.0)

            # ---- astro modulation -> ma_full [B, OS] (broadcast over batch) ----
            ctx_col = tmp.tile([B, 1], f32)
            nc.vector.tensor_reduce(
                out=ctx_col[:], in_=ctx_t[:], axis=mybir.AxisListType.X,
                op=AluOpType.add,
            )
            ctx_cs = tmp.tile([B, 1], f32)
            nc.vector.tensor_scalar_mul(ctx_cs[:], ctx_col[:], 1.0 / C)
            astro = tmp.tile([B, OS], f32)
            nc.scalar.activation(
                out=astro[:], in_=aa_r[:],
                func=mybir.ActivationFunctionType.Sigmoid, scale=ctx_cs[:],
            )
            m01 = tmp.tile([B, OS], f32)
            nc.vector.tensor_tensor(m01[:], astro[:], at_r[:], AluOpType.is_gt)
            t_ast = tmp.tile([B, OS], f32)
            nc.vector.tensor_tensor(t_ast[:], m01[:], astro[:], AluOpType.mult)
            p_ma = psmall.tile([1, OS], f32)
            nc.tensor.matmul(p_ma[:], ones64[:], t_ast[:], start=True, stop=True)
            ma_row = tmp.tile([1, OS], f32)
            nc.scalar.mul(ma_row[:], p_ma[:], 1.0 / B)
            p_mab = psmall.tile([B, OS], f32)
            nc.tensor.matmul(p_mab[:], ones1[:], ma_row[:], start=True, stop=True)
            ma_full = const.tile([B, OS], f32)
            nc.vector.tensor_copy(ma_full[:], p_mab[:])

            # ---- gate sigmoid ----
            sg = const.tile([B, S * OS], f32)
            nc.scalar.activation(
                out=sg[:], in_=gb[:], func=mybir.ActivationFunctionType.Sigmoid,
            )

            # ---- effective (pruned) weights + synaptic matmul ----
            w_full = wgt.tile([P, KT * OS], f32)
            nc.sync.dma_start(out=w_full[:].rearrange("p (k o) -> p k o", k=KT), in_=d_wt.rearrange("k p o -> p k o"))
            cs_full = wgt.tile([P, KT * OS], f32)
            nc.sync.dma_start(out=cs_full[:].rearrange("p (k o) -> p k o", k=KT), in_=d_cst.rearrange("k p o -> p k o"))
            aw_full = wgt.tile([P, KT * OS], f32)
            nc.scalar.activation(
                out=aw_full[:], in_=w_full[:],
                func=mybir.ActivationFunctionType.Abs,
            )
            # in-place: aw <- |w| * cs, then eff (into cs slot) <- (aw > thr) * w
            nc.vector.tensor_tensor(
                aw_full[:], aw_full[:], cs_full[:], AluOpType.mult,
            )
            eff = cs_full
            nc.vector.scalar_tensor_tensor(
                out=eff[:], in0=aw_full[:], scalar=thr_c[:], in1=w_full[:],
                op0=AluOpType.is_gt, op1=AluOpType.mult,
            )
            p_syn = psmall.tile([B, OS], f32)
            for kt in range(KT):
                nc.tensor.matmul(
                    p_syn[:],
                    x_tf[:, kt, :],
                    eff[:, kt * OS:(kt + 1) * OS],
                    start=(kt == 0), stop=(kt == KT - 1),
                )

            # ---- dendritic stream ----
            acc = const.tile([B, OS], f32)
            for c in range(NCH):
                seg_t = segp.tile([P, KT, NW], f32r)
                for g in range(0, KT, 4):
                    nc.sync.dma_start(
                        out=seg_t[:, g:g + 4, :],
                        in_=d_seg[c, g:g + 4].rearrange("k p j -> p k j"),
                    )
                p_d = pdp.tile([B, NW], f32)
                for kt in range(KT):
                    nc.tensor.matmul(
                        p_d[:],
                        x_tr[:, kt, :],
                        seg_t[:, kt, :],
                        start=(kt == 0), stop=(kt == KT - 1),
                    )
                gt = gtp.tile([B, NW], f32)
                nc.vector.scalar_tensor_tensor(
                    out=gt[:], in0=p_d[:], scalar=0.0,
                    in1=sg[:, c * NW:(c + 1) * NW],
                    op0=AluOpType.max, op1=AluOpType.mult,
                )
                if c == 0:
                    nc.vector.tensor_tensor(
                        acc[:], gt[:, 0:OS], gt[:, OS:NW], AluOpType.add,
                    )
                else:
                    nc.vector.tensor_tensor(
                        acc[:], acc[:], gt[:, 0:OS], AluOpType.add,
                    )
                    nc.vector.tensor_tensor(
                        acc[:], acc[:], gt[:, OS:NW], AluOpType.add,
                    )

            # ---- combine: relu(syn * ma + bias + dendritic) ----
            t1 = tmp.tile([B, OS], f32)
            nc.vector.tensor_tensor(t1[:], p_syn[:], ma_full[:], AluOpType.mult)
            t2 = tmp.tile([B, OS], f32)
            nc.vector.tensor_tensor(t2[:], t1[:], bias_r[:], AluOpType.add)
            t3 = tmp.tile([B, OS], f32)
            nc.vector.tensor_tensor(t3[:], t2[:], acc[:], AluOpType.add)
            outt = tmp.tile([B, OS], f32)
            nc.scalar.activation(
                out=outt[:], in_=t3[:], func=mybir.ActivationFunctionType.Relu,
            )
            nc.sync.dma_start(out=d_out[:], in_=outt[:])

    nc.compile()
    return nc


def _get_nc():
    global _BUILT
    if _BUILT is None:
        _BUILT = _build()
    return _BUILT


def prep_in_maps(x, context, weight, bias, astrocyte_activation,
                 astrocyte_threshold, dendrite_segments, dendritic_gates,
                 connection_strength, pruning_threshold):
    """Shard + lay out the full inputs for the 8 cores."""
    f = np.float32
    xt = np.ascontiguousarray(np.asarray(x, f).T.reshape(KT, P, B))
    ctx = np.ascontiguousarray(np.asarray(context, f))
    thr = np.asarray(pruning_threshold, f).reshape(1, 1)
    in_maps = []
    for m in range(NCORES):
        sl = slice(m * OS, (m + 1) * OS)
        wt = np.ascontiguousarray(
            np.asarray(weight[sl], f).T.reshape(KT, P, OS))
        cst = np.ascontiguousarray(
            np.asarray(connection_strength[sl], f).T.reshape(KT, P, OS))
        # [OS, I, S] -> [I, S, OS] -> columns (s-major, o-minor) -> chunked
        a = np.asarray(dendrite_segments[sl], f).transpose(1, 2, 0)
        seg = np.ascontiguousarray(
            a.reshape(I, S * OS).reshape(KT, P, NCH, NW).transpose(2, 0, 1, 3))
        gates = np.ascontiguousarray(
            np.asarray(dendritic_gates[sl], f).T.reshape(1, S * OS))
        in_maps.append({
            "xtr": xt, "xtf": xt, "ctx": ctx, "wt": wt, "cst": cst, "seg": seg,
            "gates": gates,
            "bias": np.asarray(bias[sl], f).reshape(1, OS),
            "aa": np.asarray(astrocyte_activation[sl], f).reshape(1, OS),
            "at": np.asarray(astrocyte_threshold[sl], f).reshape(1, OS),
            "thr": thr,
        })
    return in_maps


def run(in_maps, trace=False, **kwargs):
    from concourse.bass_utils import run_bass_kernel_spmd

    nc = _get_nc()
    return run_bass_kernel_spmd(
        nc, in_maps, core_ids=list(range(NCORES)), trace=trace, **kwargs
    )


def kernel(x, context, prev_activation, weight, bias, astrocyte_activation,
           astrocyte_threshold, dendrite_segments, dendritic_gates,
           connection_strength, pruning_threshold):
    in_maps = prep_in_maps(
        x, context, weight, bias, astrocyte_activation, astrocyte_threshold,
        dendrite_segments, dendritic_gates, connection_strength,
        pruning_threshold)
    res = run(in_maps)
    return np.concatenate([res.results[m]["out"] for m in range(NCORES)], axis=1)


# revision 8
# speedup vs baseline: 1.3998x; 1.3998x over previous
"""Trainium2 Bass kernel for nn_AdvancedNeuroplasticityLayer.

Reference computation (B=64, I=2048, O=2048, S=10, C=512):
  conn_mask = (|weight| * connection_strength > pruning_threshold)
  eff_w     = weight * conn_mask                                   [O, I]
  ctx       = mean(context, -1)                                    [B]
  astro     = sigmoid(ctx[:,None] * astrocyte_activation[None,:])  [B, O]
  astro    *= (astro > astrocyte_threshold)
  mean_astro = mean(astro, 0)                                      [O]
  synaptic  = x @ (eff_w * mean_astro[:,None]).T + bias            [B, O]
  seg_act   = einsum('bi,ois->bos', x, dendrite_segments)          [B, O, S]
  dendritic = sum_s relu(seg_act) * sigmoid(dendritic_gates)       [B, O]
  out       = relu(synaptic + dendritic)

Sharding: output dim O split across 8 cores (256 outputs each); x/context
replicated.  Host pre-transposes weight-like tensors to [ktile, 128, n]
(contraction dim I on partitions) so all DMAs are large & contiguous.
Matmuls run as float32r (fp32 bits at bf16 streaming rate).
"""

import os
import sys

import numpy as np

for _p in ("/opt/trn_rl_repo", "/root/.axon_site/_ro/trn_rl_repo"):
    if os.path.isdir(_p) and _p not in sys.path:
        sys.path.insert(0, _p)

B = 64
I = 2048
O = 2048
S = 10
C = 512
NCORES = 8
OS = O // NCORES  # 256 outputs per core
P = 128
KT = I // P  # 16 k-tiles
NW = 512  # matmul moving-dim chunk
NCH = (OS * S) // NW  # 5 psum chunks of the (s, o) dendritic columns

_BUILT = None


def _build():
    import concourse.tile as tile
    from concourse import bacc, mybir
    from concourse.alu_op_type import AluOpType

    f32 = mybir.dt.float32
    bf16 = mybir.dt.bfloat16

    nc = bacc.Bacc("TRN2", target_bir_lowering=False, debug=False, num_devices=NCORES)

    d_xb = nc.dram_tensor("xb", [KT, P, B], bf16, kind="ExternalInput").ap()
    d_ctx = nc.dram_tensor("ctx", [B, C], f32, kind="ExternalInput").ap()
    d_wt = nc.dram_tensor("wt", [KT, P, OS], f32, kind="ExternalInput").ap()
    d_cst = nc.dram_tensor("cst", [KT, P, OS], f32, kind="ExternalInput").ap()
    d_seg = nc.dram_tensor("seg", [NCH, KT, P, NW], bf16, kind="ExternalInput").ap()
    d_gates = nc.dram_tensor("gates", [1, S * OS], f32, kind="ExternalInput").ap()
    d_bias = nc.dram_tensor("bias", [1, OS], f32, kind="ExternalInput").ap()
    d_aa = nc.dram_tensor("aa", [1, OS], f32, kind="ExternalInput").ap()
    d_at = nc.dram_tensor("at", [1, OS], f32, kind="ExternalInput").ap()
    d_thr = nc.dram_tensor("thr", [1, 1], f32, kind="ExternalInput").ap()
    d_out = nc.dram_tensor("out", [B, OS], f32, kind="ExternalOutput").ap()

    with tile.TileContext(nc) as tc:
        with (
            tc.tile_pool(name="const", bufs=1) as const,
            tc.tile_pool(name="wgt", bufs=1) as wgt,
            tc.tile_pool(name="segp", bufs=2) as segp,
            tc.tile_pool(name="gtp", bufs=3) as gtp,
            tc.tile_pool(name="tmp", bufs=2) as tmp,
            tc.tile_pool(name="pd", bufs=2, space="PSUM") as pdp,
            tc.tile_pool(name="psmall", bufs=1, space="PSUM") as psmall,
        ):
            # ---- small input loads ----
            x_b = const.tile([P, KT, B], bf16)
            nc.sync.dma_start(out=x_b[:], in_=d_xb.rearrange("k p b -> p k b"))
            ctx_t = const.tile([B, C], f32)
            nc.sync.dma_start(out=ctx_t[:], in_=d_ctx[:])
            aa_r = const.tile([B, OS], f32)
            nc.gpsimd.dma_start(out=aa_r[:], in_=d_aa.to_broadcast([B, OS]))
            at_r = const.tile([B, OS], f32)
            nc.gpsimd.dma_start(out=at_r[:], in_=d_at.to_broadcast([B, OS]))
            bias_r = const.tile([B, OS], f32)
            nc.gpsimd.dma_start(out=bias_r[:], in_=d_bias.to_broadcast([B, OS]))
            thr_c = const.tile([P, 1], f32)
            nc.gpsimd.dma_start(out=thr_c[:], in_=d_thr.to_broadcast([P, 1]))
            gb = const.tile([B, S * OS], f32)
            nc.gpsimd.dma_start(out=gb[:], in_=d_gates.to_broadcast([B, S * OS]))

            ones64 = const.tile([B, 1], f32)
            nc.vector.memset(ones64[:], 1.0)
            ones1 = const.tile([1, B], f32)
            nc.vector.memset(ones1[:], 1.0)

            # ---- astro modulation -> ma_full [B, OS] (broadcast over batch) ----
            ctx_col = tmp.tile([B, 1], f32)
            nc.vector.tensor_reduce(
                out=ctx_col[:], in_=ctx_t[:], axis=mybir.AxisListType.X,
                op=AluOpType.add,
            )
            ctx_cs = tmp.tile([B, 1], f32)
            nc.vector.tensor_scalar_mul(ctx_cs[:], ctx_col[:], 1.0 / C)
            astro = tmp.tile([B, OS], f32)
            nc.scalar.activation(
                out=astro[:], in_=aa_r[:],
                func=mybir.ActivationFunctionType.Sigmoid, scale=ctx_cs[:],
            )
            m01 = tmp.tile([B, OS], f32)
            nc.vector.tensor_tensor(m01[:], astro[:], at_r[:], AluOpType.is_gt)
            t_ast = tmp.tile([B, OS], f32)
            nc.vector.tensor_tensor(t_ast[:], m01[:], astro[:], AluOpType.mult)
            p_ma = psmall.tile([1, OS], f32)
            nc.tensor.matmul(p_ma[:], ones64[:], t_ast[:], start=True, stop=True)
            ma_row = tmp.tile([1, OS], f32)
            nc.scalar.mul(ma_row[:], p_ma[:], 1.0 / B)
            p_mab = psmall.tile([B, OS], f32)
            nc.tensor.matmul(p_mab[:], ones1[:], ma_row[:], start=True, stop=True)
            ma_full = const.tile([B, OS], f32)
            nc.vector.tensor_copy(ma_full[:], p_mab[:])

            # ---- gate sigmoid ----
            sg = const.tile([B, S * OS], f32)
            nc.scalar.activation(
                out=sg[:], in_=gb[:], func=mybir.ActivationFunctionType.Sigmoid,
            )

            # ---- effective (pruned) weights + synaptic matmul ----
            w_full = wgt.tile([P, KT * OS], f32)
            nc.sync.dma_start(out=w_full[:].rearrange("p (k o) -> p k o", k=KT), in_=d_wt.rearrange("k p o -> p k o"))
            cs_full = wgt.tile([P, KT * OS], f32)
            nc.sync.dma_start(out=cs_full[:].rearrange("p (k o) -> p k o", k=KT), in_=d_cst.rearrange("k p o -> p k o"))
            aw_full = wgt.tile([P, KT * OS], f32)
            nc.scalar.activation(
                out=aw_full[:], in_=w_full[:],
                func=mybir.ActivationFunctionType.Abs,
            )
            # in-place: aw <- |w| * cs, then eff (into cs slot) <- (aw > thr) * w
            nc.vector.tensor_tensor(
                aw_full[:], aw_full[:], cs_full[:], AluOpType.mult,
            )
            eff = wgt.tile([P, KT * OS], bf16)
            nc.vector.scalar_tensor_tensor(
                out=eff[:], in0=aw_full[:], scalar=thr_c[:], in1=w_full[:],
                op0=AluOpType.is_gt, op1=AluOpType.mult,
            )
            p_syn = psmall.tile([B, OS], f32)
            for kt in range(KT):
                nc.tensor.matmul(
                    p_syn[:],
                    x_b[:, kt, :],
                    eff[:, kt * OS:(kt + 1) * OS],
                    start=(kt == 0), stop=(kt == KT - 1),
                )

            # ---- dendritic stream ----
            acc = const.tile([B, OS], f32)
            for c in range(NCH):
                seg_t = segp.tile([P, KT, NW], bf16)
                for g in range(0, KT, 8):
                    nc.sync.dma_start(
                        out=seg_t[:, g:g + 8, :],
                        in_=d_seg[c, g:g + 8].rearrange("k p j -> p k j"),
                    )
                p_d = pdp.tile([B, NW], f32)
                for kt in range(KT):
                    nc.tensor.matmul(
                        p_d[:],
                        x_b[:, kt, :],
                        seg_t[:, kt, :],
                        start=(kt == 0), stop=(kt == KT - 1),
                    )
                gt = gtp.tile([B, NW], f32)
                nc.vector.scalar_tensor_tensor(
                    out=gt[:], in0=p_d[:], scalar=0.0,
                    in1=sg[:, c * NW:(c + 1) * NW],
                    op0=AluOpType.max, op1=AluOpType.mult,
                )
                if c == 0:
                    nc.vector.tensor_tensor(
                        acc[:], gt[:, 0:OS], gt[:, OS:NW], AluOpType.add,
                    )
                else:
                    nc.vector.tensor_tensor(
                        acc[:], acc[:], gt[:, 0:OS], AluOpType.add,
                    )
                    nc.vector.tensor_tensor(
                        acc[:], acc[:], gt[:, OS:NW], AluOpType.add,
                    )

            # ---- combine: relu(syn * ma + bias + dendritic) ----
            t1 = tmp.tile([B, OS], f32)
            nc.vector.tensor_tensor(t1[:], p_syn[:], ma_full[:], AluOpType.mult)
            t2 = tmp.tile([B, OS], f32)
            nc.vector.tensor_tensor(t2[:], t1[:], bias_r[:], AluOpType.add)
            t3 = tmp.tile([B, OS], f32)
            nc.vector.tensor_tensor(t3[:], t2[:], acc[:], AluOpType.add)
            outt = tmp.tile([B, OS], f32)
            nc.scalar.activation(
                out=outt[:], in_=t3[:], func=mybir.ActivationFunctionType.Relu,
            )
            nc.sync.dma_start(out=d_out[:], in_=outt[:])

    nc.compile()
    return nc


def _get_nc():
    global _BUILT
    if _BUILT is None:
        _BUILT = _build()
    return _BUILT


def prep_in_maps(x, context, weight, bias, astrocyte_activation,
                 astrocyte_threshold, dendrite_segments, dendritic_gates,
                 connection_strength, pruning_threshold):
    """Shard + lay out the full inputs for the 8 cores."""
    import ml_dtypes
    f = np.float32
    bf = ml_dtypes.bfloat16
    xt = np.ascontiguousarray(np.asarray(x, f).T.reshape(KT, P, B)).astype(bf)
    ctx = np.ascontiguousarray(np.asarray(context, f))
    thr = np.asarray(pruning_threshold, f).reshape(1, 1)
    in_maps = []
    for m in range(NCORES):
        sl = slice(m * OS, (m + 1) * OS)
        wt = np.ascontiguousarray(
            np.asarray(weight[sl], f).T.reshape(KT, P, OS))
        cst = np.ascontiguousarray(
            np.asarray(connection_strength[sl], f).T.reshape(KT, P, OS))
        # [OS, I, S] -> [I, S, OS] -> columns (s-major, o-minor) -> chunked
        a = np.asarray(dendrite_segments[sl], f).transpose(1, 2, 0)
        seg = np.ascontiguousarray(
            a.reshape(I, S * OS).reshape(KT, P, NCH, NW).transpose(2, 0, 1, 3)
        ).astype(bf)
        gates = np.ascontiguousarray(
            np.asarray(dendritic_gates[sl], f).T.reshape(1, S * OS))
        in_maps.append({
            "xb": xt, "ctx": ctx, "wt": wt, "cst": cst, "seg": seg,
            "gates": gates,
            "bias": np.asarray(bias[sl], f).reshape(1, OS),
            "aa": np.asarray(astrocyte_activation[sl], f).reshape(1, OS),
            "at": np.asarray(astrocyte_threshold[sl], f).reshape(1, OS),
            "thr": thr,
        })
    return in_maps


def run(in_maps, trace=False, **kwargs):
    from concourse.bass_utils import run_bass_kernel_spmd

    nc = _get_nc()
    return run_bass_kernel_spmd(
        nc, in_maps, core_ids=list(range(NCORES)), trace=trace, **kwargs
    )


def kernel(x, context, prev_activation, weight, bias, astrocyte_activation,
           astrocyte_threshold, dendrite_segments, dendritic_gates,
           connection_strength, pruning_threshold):
    in_maps = prep_in_maps(
        x, context, weight, bias, astrocyte_activation, astrocyte_threshold,
        dendrite_segments, dendritic_gates, connection_strength,
        pruning_threshold)
    res = run(in_maps)
    return np.concatenate([res.results[m]["out"] for m in range(NCORES)], axis=1)
